# revision 7
# baseline (speedup 1.0000x reference)
"""MoE gate (group-limited top-k routing) as a Bass/Tile kernel for 8 TRN2 cores.

Computes, per token:
  logits = hidden @ W            (K=7168, E=256)
  scores = sigmoid(logits) + bias
  group-limited routing: top-2-sum per group of 32 -> top-4 groups of 8
  top-8 of masked scores, renormalized, * 2.5

Sharding: data-parallel over tokens (1024 tokens/core), W + bias replicated.

Matmul schemes:
  f16t (default): hidden tiles are pre-rounded to fp16 on the scalar/vector
    engines, PE-transposed in fp16 (1 cyc/row) into fp16 PSUM, copied back
    to SBUF (scalar/vector/gpsimd), then a single fp16 matmul per 128-K
    chunk streams W's 256 expert columns (1 cyc/row). Error ~2^-11 relative
    on logits, well within the 2e-2 gate.
  f16hi: fp32 PE transposes (2 cyc/row, no pre-round); the PSUM->SBUF
    copyback rounds to fp16; single fp16 matmul per chunk.
"""

import sys

if "/opt/trn_rl_repo" not in sys.path:
    sys.path.insert(0, "/opt/trn_rl_repo")

import numpy as np

import concourse.bacc as bacc
import concourse.bass as bass
import concourse.mybir as mybir
import concourse.tile as tile
from concourse import bass_utils
from concourse.masks import make_identity

P = 128
TOP_K = 8
N_GROUP = 8
TOPK_GROUP = 4
SCALE = 2.5

N_CORES = 8
TOKENS = 8192
HIDDEN = 7168
EXPERTS = 256

SCHEME = "f16t"


def build_moe_gate(
    tokens_per_core=TOKENS // N_CORES,
    hidden=HIDDEN,
    n_experts=EXPERTS,
    scheme=SCHEME,
):
    KC = hidden // P          # K-chunks of 128
    TT = tokens_per_core // P  # token tiles of 128
    GS = n_experts // N_GROUP  # experts per group
    BATCH = 8                  # transposes batched per PSUM copyback
    WB = 8                     # weight-load chunk batch
    f32 = mybir.dt.float32
    f16 = mybir.dt.float16

    nc = bacc.Bacc("TRN2", target_bir_lowering=False, debug=False)
    hs = nc.dram_tensor(
        "hidden_states", [tokens_per_core, hidden], f32, kind="ExternalInput"
    ).ap()
    wk = nc.dram_tensor("kernel", [hidden, n_experts], f32, kind="ExternalInput").ap()
    bias = nc.dram_tensor(
        "e_score_correction_bias", [n_experts], f32, kind="ExternalInput"
    ).ap()
    out = nc.dram_tensor(
        "topk_out", [tokens_per_core, TOP_K], f32, kind="ExternalOutput"
    ).ap()

    with tile.TileContext(nc) as tc:
        with (
            tc.tile_pool(name="const", bufs=1) as cpool,
            tc.tile_pool(name="wstage", bufs=2) as wspool,
            tc.tile_pool(name="hload", bufs=3) as hpool,
            tc.tile_pool(name="h16", bufs=2) as h16pool,
            tc.tile_pool(name="ht", bufs=4) as htpool,
            tc.tile_pool(name="ptr", bufs=4, space="PSUM") as ptpool,
            tc.tile_pool(name="plog", bufs=2, space="PSUM") as plpool,
            tc.tile_pool(name="route", bufs=2) as rpool,
        ):
            identity = cpool.tile([P, P], f32)
            make_identity(nc, identity)
            if scheme == "f16t":
                id16 = cpool.tile([P, P], f16)
                nc.vector.tensor_copy(id16, identity)

            # tile 0 gets a small first slice so the PE can start early; its
            # H slices are interleaved with the weight batches so tile-0
            # matmuls don't stall on W
            def slice_plan(t):
                if t == 0:
                    return [(0, 2), (2, 8)] + [(b, b + 8) for b in range(8, KC, 8)]
                return [(b, b + 8) for b in range(0, KC, 8)]

            wk_mm = cpool.tile([P, KC, n_experts], f16)
            wk_view = wk.rearrange("(kc p) e -> p kc e", p=P)
            w_cvt_eng = [nc.gpsimd, nc.scalar, nc.vector]

            def load_w_batch(wb):
                ws = slice(wb * WB, (wb + 1) * WB)
                wstage = wspool.tile([P, WB, n_experts], f32)
                nc.sync.dma_start(out=wstage, in_=wk_view[:, ws, :])
                eng = w_cvt_eng[wb % 3]
                if eng is nc.scalar:
                    nc.scalar.activation(
                        wk_mm[:, ws, :], wstage, mybir.ActivationFunctionType.Copy
                    )
                else:
                    eng.tensor_copy(wk_mm[:, ws, :], wstage)

            bias_sb = cpool.tile([P, n_experts], f32)

            # engine rotations for the fp32->fp16 pre-round and the
            # PSUM->SBUF copyback (keep DVE light: it also owns the epilogue)
            pr_eng = [nc.gpsimd, nc.scalar, nc.vector, nc.gpsimd,
                      nc.scalar, nc.vector, nc.gpsimd, nc.vector]
            cb_eng = [nc.scalar, nc.vector, nc.scalar, nc.vector,
                      nc.scalar, nc.vector, nc.scalar]

            for t in range(TT):
                htile = hpool.tile([P, hidden], f32)
                for i, (c0, c1) in enumerate(slice_plan(t)):
                    nc.sync.dma_start(
                        out=htile[:, c0 * P : c1 * P],
                        in_=hs[t * P : (t + 1) * P, c0 * P : c1 * P],
                    )
                    # tile 0: weight batches ride between the H slices so
                    # tile-0 matmuls don't stall on W
                    if t == 0 and i < KC // WB:
                        load_w_batch(i)
                if t == 0:
                    # bias is only needed by the first routing epilogue, well
                    # into the run; load it after the weight DMAs are queued
                    bias_bcast = bass.AP(
                        tensor=bias.tensor,
                        offset=bias.offset,
                        ap=[[0, P]] + list(bias.ap),
                    )
                    nc.gpsimd.dma_start(out=bias_sb, in_=bias_bcast)

                if scheme == "f16t":
                    h16 = h16pool.tile([P, hidden], f16)
                    for i, (c0, c1) in enumerate(slice_plan(t)):
                        eng = pr_eng[i % len(pr_eng)]
                        if eng is nc.scalar:
                            nc.scalar.activation(
                                h16[:, c0 * P : c1 * P],
                                htile[:, c0 * P : c1 * P],
                                mybir.ActivationFunctionType.Copy,
                            )
                        else:
                            eng.tensor_copy(
                                h16[:, c0 * P : c1 * P], htile[:, c0 * P : c1 * P]
                            )
                    tsrc, tdt = h16, f16
                    tident = id16
                else:
                    tsrc, tdt = htile, f32
                    tident = identity

                logits_ps = plpool.tile([P, n_experts], f32)

                n_mm = 0
                for b in range(KC // BATCH):
                    tp = ptpool.tile([P, BATCH * P], tdt)
                    for j in range(BATCH):
                        k = b * BATCH + j
                        nc.tensor.transpose(
                            tp[:, j * P : (j + 1) * P],
                            tsrc[:, k * P : (k + 1) * P],
                            tident,
                        )
                    hT = htpool.tile([P, BATCH * P], f16)
                    eng = cb_eng[b % len(cb_eng)]
                    if eng is nc.scalar:
                        nc.scalar.activation(
                            hT, tp, mybir.ActivationFunctionType.Copy
                        )
                    else:
                        eng.tensor_copy(hT, tp)
                    for j in range(BATCH):
                        k = b * BATCH + j
                        nc.tensor.matmul(
                            logits_ps,
                            lhsT=hT[:, j * P : (j + 1) * P],
                            rhs=wk_mm[:, k, :],
                            start=(n_mm == 0),
                            stop=(n_mm == KC - 1),
                        )
                        n_mm += 1

                # ---- routing epilogue (tokens on partitions) ----
                sc = rpool.tile([P, n_experts], f32)
                nc.scalar.activation(
                    sc, logits_ps, mybir.ActivationFunctionType.Sigmoid
                )
                nc.vector.tensor_add(sc, sc, bias_sb)

                # top-2 sum per group of GS experts
                m8 = rpool.tile([P, N_GROUP * 8], f32)
                for g in range(N_GROUP):
                    nc.vector.max(
                        m8[:, g * 8 : (g + 1) * 8], sc[:, g * GS : (g + 1) * GS]
                    )
                m8v = m8.rearrange("p (g k) -> p g k", k=8)
                gsum = rpool.tile([P, N_GROUP], f32)
                nc.vector.tensor_add(gsum, m8v[:, :, 0], m8v[:, :, 1])

                # top-TOPK_GROUP groups -> per-group 0/1 mask via threshold
                gmax = rpool.tile([P, 8], f32)
                nc.vector.max(gmax, gsum)
                gmask = rpool.tile([P, N_GROUP], f32)
                nc.vector.tensor_scalar(
                    gmask,
                    gsum,
                    gmax[:, TOPK_GROUP - 1 : TOPK_GROUP],
                    None,
                    op0=mybir.AluOpType.is_ge,
                )

                # masked scores = sc * mask (0 where group dropped)
                masked = rpool.tile([P, n_experts], f32)
                nc.vector.tensor_mul(
                    masked.rearrange("p (g e) -> p g e", g=N_GROUP),
                    sc.rearrange("p (g e) -> p g e", g=N_GROUP),
                    gmask[:, :, None].broadcast_to([P, N_GROUP, GS]),
                )

                top8 = rpool.tile([P, TOP_K], f32)
                nc.vector.max(top8, masked)

                dsum = rpool.tile([P, 1], f32)
                nc.vector.reduce_sum(dsum, top8, axis=mybir.AxisListType.X)
                rcp = rpool.tile([P, 1], f32)
                nc.vector.reciprocal(rcp, dsum)
                wout = rpool.tile([P, TOP_K], f32)
                nc.vector.tensor_scalar(
                    wout,
                    top8,
                    rcp,
                    SCALE,
                    op0=mybir.AluOpType.mult,
                    op1=mybir.AluOpType.mult,
                )
                nc.sync.dma_start(out=out[t * P : (t + 1) * P, :], in_=wout)

    nc.compile()
    return nc


_CACHE = {}


def _built_nc():
    if "nc" not in _CACHE:
        _CACHE["nc"] = build_moe_gate()
    return _CACHE["nc"]


def kernel(hidden_states, kernel, e_score_correction_bias):
    hs = np.ascontiguousarray(np.asarray(hidden_states), dtype=np.float32)
    wk = np.ascontiguousarray(np.asarray(kernel), dtype=np.float32)
    bi = np.ascontiguousarray(np.asarray(e_score_correction_bias), dtype=np.float32)
    assert hs.shape == (TOKENS, HIDDEN) and wk.shape == (HIDDEN, EXPERTS)

    tpc = TOKENS // N_CORES
    nc = _built_nc()
    in_maps = [
        {
            "hidden_states": hs[i * tpc : (i + 1) * tpc],
            "kernel": wk,
            "e_score_correction_bias": bi,
        }
        for i in range(N_CORES)
    ]
    res = bass_utils.run_bass_kernel_spmd(nc, in_maps, core_ids=list(range(N_CORES)))
    return np.concatenate(
        [res.results[i]["topk_out"] for i in range(N_CORES)], axis=0
    )


# revision 9
# speedup vs baseline: 1.2445x; 1.2445x over previous
"""MoE gate (group-limited top-k routing) as a Bass/Tile kernel for 8 TRN2 cores.

Computes, per token:
  logits = hidden @ W            (K=7168, E=256)
  scores = sigmoid(logits) + bias
  group-limited routing: top-2-sum per group of 32 -> top-4 groups of 8
  top-8 of masked scores, renormalized, * 2.5

Sharding: data-parallel over tokens (1024 tokens/core), W + bias replicated.

Matmul schemes:
  f16t (default): hidden tiles are pre-rounded to fp16 on the scalar/vector
    engines, PE-transposed in fp16 (1 cyc/row) into fp16 PSUM, copied back
    to SBUF (scalar/vector/gpsimd), then a single fp16 matmul per 128-K
    chunk streams W's 256 expert columns (1 cyc/row). Error ~2^-11 relative
    on logits, well within the 2e-2 gate.
  f16hi: fp32 PE transposes (2 cyc/row, no pre-round); the PSUM->SBUF
    copyback rounds to fp16; single fp16 matmul per chunk.
"""

import sys

if "/opt/trn_rl_repo" not in sys.path:
    sys.path.insert(0, "/opt/trn_rl_repo")

import numpy as np

import concourse.bacc as bacc
import concourse.bass as bass
import concourse.mybir as mybir
import concourse.tile as tile
from concourse import bass_utils
from concourse.masks import make_identity

P = 128
TOP_K = 8
N_GROUP = 8
TOPK_GROUP = 4
SCALE = 2.5

N_CORES = 8
TOKENS = 8192
HIDDEN = 7168
EXPERTS = 256

SCHEME = "f16hi"


def build_moe_gate(
    tokens_per_core=TOKENS // N_CORES,
    hidden=HIDDEN,
    n_experts=EXPERTS,
    scheme=SCHEME,
):
    KC = hidden // P          # K-chunks of 128
    TT = tokens_per_core // P  # token tiles of 128
    GS = n_experts // N_GROUP  # experts per group
    BATCH = 8                  # transposes batched per PSUM copyback
    WB = 8                     # weight-load chunk batch
    f32 = mybir.dt.float32
    f16 = mybir.dt.float16

    nc = bacc.Bacc("TRN2", target_bir_lowering=False, debug=False)
    hs = nc.dram_tensor(
        "hidden_states", [tokens_per_core, hidden], f32, kind="ExternalInput"
    ).ap()
    wk = nc.dram_tensor("kernel", [hidden, n_experts], f32, kind="ExternalInput").ap()
    bias = nc.dram_tensor(
        "e_score_correction_bias", [n_experts], f32, kind="ExternalInput"
    ).ap()
    out = nc.dram_tensor(
        "topk_out", [tokens_per_core, TOP_K], f32, kind="ExternalOutput"
    ).ap()

    with tile.TileContext(nc) as tc:
        with (
            tc.tile_pool(name="const", bufs=1) as cpool,
            tc.tile_pool(name="wstage", bufs=2) as wspool,
            tc.tile_pool(name="hload", bufs=3) as hpool,
            tc.tile_pool(name="h16", bufs=2) as h16pool,
            tc.tile_pool(name="ht", bufs=4) as htpool,
            tc.tile_pool(
                name="ptr", bufs=4 if scheme == "f16t" else 3, space="PSUM"
            ) as ptpool,
            tc.tile_pool(name="plog", bufs=2, space="PSUM") as plpool,
            tc.tile_pool(name="route", bufs=2) as rpool,
        ):
            identity = cpool.tile([P, P], f32)
            make_identity(nc, identity)
            if scheme == "f16t":
                id16 = cpool.tile([P, P], f16)
                nc.vector.tensor_copy(id16, identity)

            # tile 0 gets a small first slice so the PE can start early; its
            # H slices are interleaved with the weight batches so tile-0
            # matmuls don't stall on W
            def slice_plan(t):
                if t == 0:
                    return [(0, 2), (2, 8)] + [(b, b + 8) for b in range(8, KC, 8)]
                return [(b, b + 8) for b in range(0, KC, 8)]

            wk_mm = cpool.tile([P, KC, n_experts], f16)
            wk_view = wk.rearrange("(kc p) e -> p kc e", p=P)
            w_cvt_eng = [nc.gpsimd, nc.scalar, nc.vector]

            def load_w_batch(wb):
                ws = slice(wb * WB, (wb + 1) * WB)
                wstage = wspool.tile([P, WB, n_experts], f32)
                nc.sync.dma_start(out=wstage, in_=wk_view[:, ws, :])
                eng = w_cvt_eng[wb % 3]
                if eng is nc.scalar:
                    nc.scalar.activation(
                        wk_mm[:, ws, :], wstage, mybir.ActivationFunctionType.Copy
                    )
                else:
                    eng.tensor_copy(wk_mm[:, ws, :], wstage)

            bias_sb = cpool.tile([P, n_experts], f32)

            # engine rotations for the fp32->fp16 pre-round and the
            # PSUM->SBUF copyback (keep DVE light: it also owns the epilogue)
            pr_eng = [nc.gpsimd, nc.scalar, nc.vector, nc.gpsimd,
                      nc.scalar, nc.vector, nc.gpsimd, nc.vector]
            cb_eng = [nc.scalar, nc.vector, nc.scalar, nc.vector,
                      nc.scalar, nc.vector, nc.scalar]

            for t in range(TT):
                htile = hpool.tile([P, hidden], f32)
                for i, (c0, c1) in enumerate(slice_plan(t)):
                    nc.sync.dma_start(
                        out=htile[:, c0 * P : c1 * P],
                        in_=hs[t * P : (t + 1) * P, c0 * P : c1 * P],
                    )
                    # tile 0: weight batches ride between the H slices so
                    # tile-0 matmuls don't stall on W
                    if t == 0 and i < KC // WB:
                        load_w_batch(i)
                if t == 0:
                    # bias is only needed by the first routing epilogue, well
                    # into the run; load it after the weight DMAs are queued
                    bias_bcast = bass.AP(
                        tensor=bias.tensor,
                        offset=bias.offset,
                        ap=[[0, P]] + list(bias.ap),
                    )
                    nc.gpsimd.dma_start(out=bias_sb, in_=bias_bcast)

                if scheme == "f16t":
                    h16 = h16pool.tile([P, hidden], f16)
                    for i, (c0, c1) in enumerate(slice_plan(t)):
                        eng = pr_eng[i % len(pr_eng)]
                        if eng is nc.scalar:
                            nc.scalar.activation(
                                h16[:, c0 * P : c1 * P],
                                htile[:, c0 * P : c1 * P],
                                mybir.ActivationFunctionType.Copy,
                            )
                        else:
                            eng.tensor_copy(
                                h16[:, c0 * P : c1 * P], htile[:, c0 * P : c1 * P]
                            )
                    tsrc, tdt = h16, f16
                    tident = id16
                else:
                    tsrc, tdt = htile, f32
                    tident = identity

                logits_ps = plpool.tile([P, n_experts], f32)

                n_mm = 0
                for b in range(KC // BATCH):
                    tp = ptpool.tile([P, BATCH * P], tdt)
                    for j in range(BATCH):
                        k = b * BATCH + j
                        nc.tensor.transpose(
                            tp[:, j * P : (j + 1) * P],
                            tsrc[:, k * P : (k + 1) * P],
                            tident,
                        )
                    hT = htpool.tile([P, BATCH * P], f16)
                    eng = cb_eng[b % len(cb_eng)]
                    if eng is nc.scalar:
                        nc.scalar.activation(
                            hT, tp, mybir.ActivationFunctionType.Copy
                        )
                    else:
                        eng.tensor_copy(hT, tp)
                    for j in range(BATCH):
                        k = b * BATCH + j
                        nc.tensor.matmul(
                            logits_ps,
                            lhsT=hT[:, j * P : (j + 1) * P],
                            rhs=wk_mm[:, k, :],
                            start=(n_mm == 0),
                            stop=(n_mm == KC - 1),
                        )
                        n_mm += 1

                # ---- routing epilogue (tokens on partitions) ----
                sc = rpool.tile([P, n_experts], f32)
                nc.scalar.activation(
                    sc, logits_ps, mybir.ActivationFunctionType.Sigmoid
                )
                nc.vector.tensor_add(sc, sc, bias_sb)

                # top-2 sum per group of GS experts
                m8 = rpool.tile([P, N_GROUP * 8], f32)
                for g in range(N_GROUP):
                    nc.vector.max(
                        m8[:, g * 8 : (g + 1) * 8], sc[:, g * GS : (g + 1) * GS]
                    )
                m8v = m8.rearrange("p (g k) -> p g k", k=8)
                gsum = rpool.tile([P, N_GROUP], f32)
                nc.vector.tensor_add(gsum, m8v[:, :, 0], m8v[:, :, 1])

                # top-TOPK_GROUP groups -> per-group 0/1 mask via threshold
                gmax = rpool.tile([P, 8], f32)
                nc.vector.max(gmax, gsum)
                gmask = rpool.tile([P, N_GROUP], f32)
                nc.vector.tensor_scalar(
                    gmask,
                    gsum,
                    gmax[:, TOPK_GROUP - 1 : TOPK_GROUP],
                    None,
                    op0=mybir.AluOpType.is_ge,
                )

                # masked scores = sc * mask (0 where group dropped)
                masked = rpool.tile([P, n_experts], f32)
                nc.vector.tensor_mul(
                    masked.rearrange("p (g e) -> p g e", g=N_GROUP),
                    sc.rearrange("p (g e) -> p g e", g=N_GROUP),
                    gmask[:, :, None].broadcast_to([P, N_GROUP, GS]),
                )

                top8 = rpool.tile([P, TOP_K], f32)
                nc.vector.max(top8, masked)

                dsum = rpool.tile([P, 1], f32)
                nc.vector.reduce_sum(dsum, top8, axis=mybir.AxisListType.X)
                rcp = rpool.tile([P, 1], f32)
                nc.vector.reciprocal(rcp, dsum)
                wout = rpool.tile([P, TOP_K], f32)
                nc.vector.tensor_scalar(
                    wout,
                    top8,
                    rcp,
                    SCALE,
                    op0=mybir.AluOpType.mult,
                    op1=mybir.AluOpType.mult,
                )
                nc.sync.dma_start(out=out[t * P : (t + 1) * P, :], in_=wout)

    nc.compile()
    return nc


_CACHE = {}


def _built_nc():
    if "nc" not in _CACHE:
        _CACHE["nc"] = build_moe_gate()
    return _CACHE["nc"]


def kernel(hidden_states, kernel, e_score_correction_bias):
    hs = np.ascontiguousarray(np.asarray(hidden_states), dtype=np.float32)
    wk = np.ascontiguousarray(np.asarray(kernel), dtype=np.float32)
    bi = np.ascontiguousarray(np.asarray(e_score_correction_bias), dtype=np.float32)
    assert hs.shape == (TOKENS, HIDDEN) and wk.shape == (HIDDEN, EXPERTS)

    tpc = TOKENS // N_CORES
    nc = _built_nc()
    in_maps = [
        {
            "hidden_states": hs[i * tpc : (i + 1) * tpc],
            "kernel": wk,
            "e_score_correction_bias": bi,
        }
        for i in range(N_CORES)
    ]
    res = bass_utils.run_bass_kernel_spmd(nc, in_maps, core_ids=list(range(N_CORES)))
    return np.concatenate(
        [res.results[i]["topk_out"] for i in range(N_CORES)], axis=0
    )


# revision 11
# speedup vs baseline: 1.3358x; 1.0734x over previous
"""MoE gate (group-limited top-k routing) as a Bass/Tile kernel for 8 TRN2 cores.

Computes, per token:
  logits = hidden @ W            (K=7168, E=256)
  scores = sigmoid(logits) + bias
  group-limited routing: top-2-sum per group of 32 -> top-4 groups of 8
  top-8 of masked scores, renormalized, * 2.5

Sharding: data-parallel over tokens (1024 tokens/core), W + bias replicated.

Matmul schemes:
  f16t (default): hidden tiles are pre-rounded to fp16 on the scalar/vector
    engines, PE-transposed in fp16 (1 cyc/row) into fp16 PSUM, copied back
    to SBUF (scalar/vector/gpsimd), then a single fp16 matmul per 128-K
    chunk streams W's 256 expert columns (1 cyc/row). Error ~2^-11 relative
    on logits, well within the 2e-2 gate.
  f16hi: fp32 PE transposes (2 cyc/row, no pre-round); the PSUM->SBUF
    copyback rounds to fp16; single fp16 matmul per chunk.
"""

import sys

if "/opt/trn_rl_repo" not in sys.path:
    sys.path.insert(0, "/opt/trn_rl_repo")

import numpy as np

import concourse.bacc as bacc
import concourse.bass as bass
import concourse.mybir as mybir
import concourse.tile as tile
from concourse import bass_utils
from concourse.masks import make_identity

P = 128
TOP_K = 8
N_GROUP = 8
TOPK_GROUP = 4
SCALE = 2.5

N_CORES = 8
TOKENS = 8192
HIDDEN = 7168
EXPERTS = 256

SCHEME = "f16hi"


def build_moe_gate(
    tokens_per_core=TOKENS // N_CORES,
    hidden=HIDDEN,
    n_experts=EXPERTS,
    scheme=SCHEME,
):
    KC = hidden // P          # K-chunks of 128
    TT = tokens_per_core // P  # token tiles of 128
    GS = n_experts // N_GROUP  # experts per group
    BATCH = 8                  # transposes batched per PSUM copyback
    WB = 8                     # weight-load chunk batch
    f32 = mybir.dt.float32
    f16 = mybir.dt.float16

    nc = bacc.Bacc("TRN2", target_bir_lowering=False, debug=False)
    hs = nc.dram_tensor(
        "hidden_states", [tokens_per_core, hidden], f32, kind="ExternalInput"
    ).ap()
    wk = nc.dram_tensor("kernel", [hidden, n_experts], f32, kind="ExternalInput").ap()
    bias = nc.dram_tensor(
        "e_score_correction_bias", [n_experts], f32, kind="ExternalInput"
    ).ap()
    out = nc.dram_tensor(
        "topk_out", [tokens_per_core, TOP_K], f32, kind="ExternalOutput"
    ).ap()

    with tile.TileContext(nc) as tc:
        with (
            tc.tile_pool(name="const", bufs=1) as cpool,
            tc.tile_pool(name="wstage", bufs=2) as wspool,
            tc.tile_pool(name="hload", bufs=3) as hpool,
            tc.tile_pool(name="h16", bufs=2) as h16pool,
            tc.tile_pool(name="ht", bufs=4) as htpool,
            tc.tile_pool(
                name="ptr", bufs=4 if scheme == "f16t" else 3, space="PSUM"
            ) as ptpool,
            tc.tile_pool(name="plog", bufs=2, space="PSUM") as plpool,
            tc.tile_pool(name="route", bufs=2) as rpool,
        ):
            identity = cpool.tile([P, P], f32)
            make_identity(nc, identity)
            if scheme == "f16t":
                id16 = cpool.tile([P, P], f16)
                nc.vector.tensor_copy(id16, identity)

            # tile 0 gets a small first slice so the PE can start early; its
            # H slices are interleaved with the weight batches so tile-0
            # matmuls don't stall on W
            def slice_plan(t):
                if t == 0:
                    return [(0, 2), (2, 8)] + [(b, b + 8) for b in range(8, KC, 8)]
                return [(b, b + 8) for b in range(0, KC, 8)]

            wk_mm = cpool.tile([P, KC, n_experts], f16)
            wk_view = wk.rearrange("(kc p) e -> p kc e", p=P)
            w_cvt_eng = [nc.scalar, nc.vector]

            def load_w_batch(wb):
                ws = slice(wb * WB, (wb + 1) * WB)
                wstage = wspool.tile([P, WB, n_experts], f32)
                nc.sync.dma_start(out=wstage, in_=wk_view[:, ws, :])
                eng = w_cvt_eng[wb % 2]
                if eng is nc.scalar:
                    nc.scalar.activation(
                        wk_mm[:, ws, :], wstage, mybir.ActivationFunctionType.Copy
                    )
                else:
                    eng.tensor_copy(wk_mm[:, ws, :], wstage)

            bias_sb = cpool.tile([P, n_experts], f32)

            # engine rotations for the fp32->fp16 pre-round and the
            # PSUM->SBUF copyback (keep DVE light: it also owns the epilogue)
            pr_eng = [nc.gpsimd, nc.scalar, nc.vector, nc.gpsimd,
                      nc.scalar, nc.vector, nc.gpsimd, nc.vector]
            cb_eng = [nc.scalar, nc.vector, nc.scalar, nc.vector,
                      nc.scalar, nc.vector, nc.scalar]

            for t in range(TT):
                htile = hpool.tile([P, hidden], f32)
                for i, (c0, c1) in enumerate(slice_plan(t)):
                    nc.sync.dma_start(
                        out=htile[:, c0 * P : c1 * P],
                        in_=hs[t * P : (t + 1) * P, c0 * P : c1 * P],
                    )
                    # tile 0: weight batches ride between the H slices so
                    # tile-0 matmuls don't stall on W
                    if t == 0 and i < KC // WB:
                        load_w_batch(i)
                if t == 0:
                    # bias is only needed by the first routing epilogue, well
                    # into the run; load it after the weight DMAs are queued
                    bias_bcast = bass.AP(
                        tensor=bias.tensor,
                        offset=bias.offset,
                        ap=[[0, P]] + list(bias.ap),
                    )
                    nc.gpsimd.dma_start(out=bias_sb, in_=bias_bcast)

                if scheme == "f16t":
                    h16 = h16pool.tile([P, hidden], f16)
                    for i, (c0, c1) in enumerate(slice_plan(t)):
                        eng = pr_eng[i % len(pr_eng)]
                        if eng is nc.scalar:
                            nc.scalar.activation(
                                h16[:, c0 * P : c1 * P],
                                htile[:, c0 * P : c1 * P],
                                mybir.ActivationFunctionType.Copy,
                            )
                        else:
                            eng.tensor_copy(
                                h16[:, c0 * P : c1 * P], htile[:, c0 * P : c1 * P]
                            )
                    tsrc, tdt = h16, f16
                    tident = id16
                else:
                    tsrc, tdt = htile, f32
                    tident = identity

                logits_ps = plpool.tile([P, n_experts], f32)

                n_mm = 0
                for b in range(KC // BATCH):
                    tp = ptpool.tile([P, BATCH * P], tdt)
                    for j in range(BATCH):
                        k = b * BATCH + j
                        nc.tensor.transpose(
                            tp[:, j * P : (j + 1) * P],
                            tsrc[:, k * P : (k + 1) * P],
                            tident,
                        )
                    hT = htpool.tile([P, BATCH * P], f16)
                    eng = cb_eng[b % len(cb_eng)]
                    if eng is nc.scalar:
                        nc.scalar.activation(
                            hT, tp, mybir.ActivationFunctionType.Copy
                        )
                    else:
                        eng.tensor_copy(hT, tp)
                    for j in range(BATCH):
                        k = b * BATCH + j
                        nc.tensor.matmul(
                            logits_ps,
                            lhsT=hT[:, j * P : (j + 1) * P],
                            rhs=wk_mm[:, k, :],
                            start=(n_mm == 0),
                            stop=(n_mm == KC - 1),
                        )
                        n_mm += 1

                # ---- routing epilogue (tokens on partitions) ----
                sc = rpool.tile([P, n_experts], f32)
                nc.scalar.activation(
                    sc, logits_ps, mybir.ActivationFunctionType.Sigmoid
                )
                nc.vector.tensor_add(sc, sc, bias_sb)

                # top-2 sum per group of GS experts
                m8 = rpool.tile([P, N_GROUP * 8], f32)
                for g in range(N_GROUP):
                    nc.vector.max(
                        m8[:, g * 8 : (g + 1) * 8], sc[:, g * GS : (g + 1) * GS]
                    )
                m8v = m8.rearrange("p (g k) -> p g k", k=8)
                gsum = rpool.tile([P, N_GROUP], f32)
                nc.vector.tensor_add(gsum, m8v[:, :, 0], m8v[:, :, 1])

                # top-TOPK_GROUP groups -> per-group 0/1 mask via threshold
                gmax = rpool.tile([P, 8], f32)
                nc.vector.max(gmax, gsum)
                gmask = rpool.tile([P, N_GROUP], f32)
                nc.vector.tensor_scalar(
                    gmask,
                    gsum,
                    gmax[:, TOPK_GROUP - 1 : TOPK_GROUP],
                    None,
                    op0=mybir.AluOpType.is_ge,
                )

                # masked scores = sc * mask (0 where group dropped)
                masked = rpool.tile([P, n_experts], f32)
                nc.vector.tensor_mul(
                    masked.rearrange("p (g e) -> p g e", g=N_GROUP),
                    sc.rearrange("p (g e) -> p g e", g=N_GROUP),
                    gmask[:, :, None].broadcast_to([P, N_GROUP, GS]),
                )

                top8 = rpool.tile([P, TOP_K], f32)
                nc.vector.max(top8, masked)

                dsum = rpool.tile([P, 1], f32)
                nc.vector.reduce_sum(dsum, top8, axis=mybir.AxisListType.X)
                rcp = rpool.tile([P, 1], f32)
                nc.vector.reciprocal(rcp, dsum)
                wout = rpool.tile([P, TOP_K], f32)
                nc.vector.tensor_scalar(
                    wout,
                    top8,
                    rcp,
                    SCALE,
                    op0=mybir.AluOpType.mult,
                    op1=mybir.AluOpType.mult,
                )
                nc.sync.dma_start(out=out[t * P : (t + 1) * P, :], in_=wout)

    nc.compile()
    return nc


_CACHE = {}


def _built_nc():
    if "nc" not in _CACHE:
        _CACHE["nc"] = build_moe_gate()
    return _CACHE["nc"]


def kernel(hidden_states, kernel, e_score_correction_bias):
    hs = np.ascontiguousarray(np.asarray(hidden_states), dtype=np.float32)
    wk = np.ascontiguousarray(np.asarray(kernel), dtype=np.float32)
    bi = np.ascontiguousarray(np.asarray(e_score_correction_bias), dtype=np.float32)
    assert hs.shape == (TOKENS, HIDDEN) and wk.shape == (HIDDEN, EXPERTS)

    tpc = TOKENS // N_CORES
    nc = _built_nc()
    in_maps = [
        {
            "hidden_states": hs[i * tpc : (i + 1) * tpc],
            "kernel": wk,
            "e_score_correction_bias": bi,
        }
        for i in range(N_CORES)
    ]
    res = bass_utils.run_bass_kernel_spmd(nc, in_maps, core_ids=list(range(N_CORES)))
    return np.concatenate(
        [res.results[i]["topk_out"] for i in range(N_CORES)], axis=0
    )
